# revision 21
# baseline (speedup 1.0000x reference)
"""LIF layer (T=64, B=128, 2048->2048) on 8 trn2 NeuronCores.

Strategy: hybrid sharding, 4-way over batch x 2-way over out_dim.
Each core owns 32 batches and 1024 output channels, so per-core HBM
traffic is x:16MB + W:8.4MB + out:4.2MB -- under the f32r matmul floor
(262144 rows @ 1 cyc/row ~ 109us). Single-pass f32r GEMM (measured
spike rel-err ~1.5e-2, gate is 2e-2; the error is the PE's internal
fp22 truncation, operand pre-rounding does not change it).

Per core:
  GEMM  cur[o, t*32+b] = sum_i W[o,i] x[t,b,i]  (f32r, 8 ot-tiles of 128)
  SCAN  64 LIF steps on [128, 8, 32] tiles (DVE), 3 fused ops/step,
        reading cur straight out of a 4-bank PSUM tile (strided AP).
Bias folded into per-channel threshold/init via u = mem - b/(1-decay).

DMA layout notes (per-partition line size drives DMA rate):
  x   packed per 256-col block -> one DMA of 16KB/partition lines
  W   4 k-chunks of 16KB/partition lines; block0's x prefetched ahead
      of W on the same queue so the first matmul isn't starved
  out bf16 spikes, one slab DMA per block (4KB/partition lines),
      dram laid out [128, T, NOT, BL] so slabs are contiguous
"""

import math
import os

import numpy as np

import concourse.bacc as bacc
import concourse.bass as bass
import concourse.mybir as mybir
import concourse.tile as tile
from concourse import bass_utils

# Problem constants (hardcoded per contract)
T, B, I, O = 64, 128, 2048, 2048
N_CORES = 8
GRID = os.environ.get("LIF_GRID", "4x2")
BG, OG = (4, 2) if GRID == "4x2" else (2, 4)  # batch-groups x out-groups
BL = B // BG               # batches per core
OL = O // OG               # out-channels per core
NOT = OL // 128            # ot-tiles
TBc = T * BL               # columns per core
KT = I // 128              # 16 k-tiles
BLK = 256 if GRID == "4x2" else 512  # columns per block (= 8 timesteps)
SPB = BLK // BL            # timesteps per block
N_BLK = TBc // BLK         # 8 blocks
OTPB = max(1, 512 // BLK)  # ot-tiles sharing one psum bank
TAU, THR = 2.0, 1.0
DECAY = math.exp(-1.0 / TAU)

F32 = mybir.dt.float32
F32R = mybir.dt.float32r
BF16 = mybir.dt.bfloat16
ALU = mybir.AluOpType

MODE = f"f32r_{GRID}_v4"

_cache = {}


def _build_nc():
    nc = bacc.Bacc(trn_type="TRN2", target_bir_lowering=False)

    # x packed per block: [N_BLK, 128(p), KT, BLK] -> 16KB/partition lines
    xP = nc.dram_tensor("xP", [N_BLK, 128, KT, BLK], F32R, kind="ExternalInput")
    w_packed = nc.dram_tensor("w_packed", [128, KT, NOT, 128], F32R,
                              kind="ExternalInput")
    thr_d = nc.dram_tensor("thr", [128, NOT, BL], F32, kind="ExternalInput")
    u0_d = nc.dram_tensor("u0", [128, NOT, BL], F32, kind="ExternalInput")
    # t-major per partition so a block's 8 steps are one contiguous slab
    out_d = nc.dram_tensor("out", [128, T, NOT, BL], BF16, kind="ExternalOutput")

    with tile.TileContext(nc) as tc:
        with (
            tc.tile_pool(name="wpool", bufs=1) as wpool,
            tc.tile_pool(name="xpool", bufs=3) as xpool,
            tc.tile_pool(name="state", bufs=1) as state,
            tc.tile_pool(name="spkpool", bufs=2) as spkpool,
            tc.tile_pool(name="psum", bufs=2, space="PSUM") as psum_pool,
        ):
            # The sync queue spins up first (~8.6us vs ~12us for the
            # others), so the critical W + x0 stream rides it alone,
            # interleaved by k-chunk so the first matmuls gate on ~2.6MB.
            # The next block(s)' x follows on the SAME queue: FIFO order
            # keeps those prefetches from stealing bandwidth from W.
            HS = 2 if GRID == "4x2" else 1   # head blocks prefetched on sync
            w_all = wpool.tile([128, KT, NOT, 128], F32R)
            xt0 = xpool.tile([128, KT, BLK], F32R, tag="xt", name="xt_0")
            for wc in range(4):
                nc.sync.dma_start(w_all[:, 4 * wc:4 * wc + 4],
                                  w_packed[:, 4 * wc:4 * wc + 4])
                nc.sync.dma_start(xt0[:, 4 * wc:4 * wc + 4],
                                  xP[0][:, 4 * wc:4 * wc + 4])
            head_xts = {}
            for blk in range(1, HS + 1):
                xt = xpool.tile([128, KT, BLK], F32R, tag="xt",
                                name=f"xt_{blk}")
                nc.sync.dma_start(xt[:], xP[blk])
                head_xts[blk] = xt

            # also WARMS the scalar/gpsimd DMA queues: a queue's first use
            # pays ~12us of lazy ring init, so give each one a tiny DMA at
            # the head rather than letting a mid-run x prefetch eat it.
            u = state.tile([128, NOT, BL], F32)
            thr_t = state.tile([128, NOT, BL], F32)
            nc.scalar.dma_start(u[:], u0_d[:])
            nc.gpsimd.dma_start(thr_t[:], thr_d[:])

            for blk in range(N_BLK):
                if blk == 0:
                    xt = xt0
                elif blk in head_xts:
                    xt = head_xts[blk]
                else:
                    xt = xpool.tile([128, KT, BLK], F32R, tag="xt",
                                    name=f"xt_{blk}")
                    # alternate queues so consecutive prefetches overlap
                    eng = nc.scalar if blk % 2 else nc.gpsimd
                    eng.dma_start(xt[:], xP[blk])
                # 4-bank psum tile; each matmul dst sits in a bank half
                ps = psum_pool.tile([128, NOT, BLK], F32, tag="ps",
                                    name=f"ps_{blk}")
                for k in range(KT):
                    for ot in range(NOT):
                        # start=True zeroes the WHOLE psum bank; when a
                        # bank holds several ot tiles only the first one
                        # starts -- the others land on the bank-wide zero.
                        nc.tensor.matmul(
                            ps[:, ot, :],
                            w_all[:, k, ot, :],
                            xt[:, k, :],
                            start=(k == 0 and ot % OTPB == 0),
                            stop=(k == KT - 1),
                            skip_group_check=(ot % OTPB != 0),
                        )

                # LIF steps consuming this block's PSUM
                last_blk = blk == N_BLK - 1
                spk_blk = spkpool.tile([128, SPB, NOT, BL], BF16, tag="spk")
                for tl in range(SPB):
                    cur = ps[:, :, tl * BL:(tl + 1) * BL]  # [128, 8, 32]
                    nc.vector.scalar_tensor_tensor(
                        u[:], u[:], DECAY, cur, op0=ALU.mult, op1=ALU.add)
                    spk = spk_blk[:, tl]
                    nc.vector.tensor_tensor(spk, u[:], thr_t[:], op=ALU.is_gt)
                    if not (last_blk and tl == SPB - 1):
                        # u after the final step is unused
                        nc.vector.tensor_tensor(u[:], u[:], spk, op=ALU.subtract)
                    if last_blk:
                        # per-step DMAs so the final drain is one tiny
                        # transfer instead of a whole slab
                        nc.sync.dma_start(out_d[:, blk * SPB + tl], spk)
                if not last_blk:
                    # one slab DMA per block: [128, 8, NOT, BL] -> 4KB
                    # lines on the sync queue, which is idle after the head
                    nc.sync.dma_start(
                        out_d[:, blk * SPB:(blk + 1) * SPB], spk_blk[:])

    nc.compile()
    return nc


def _get_nc():
    if "nc" not in _cache:
        _cache["nc"] = _build_nc()
    return _cache["nc"]


def kernel(x_seq: np.ndarray, W: np.ndarray, b: np.ndarray) -> np.ndarray:
    nc = _get_nc()

    x_seq = np.asarray(x_seq, np.float32)
    W = np.asarray(W, np.float32)
    b = np.asarray(b, np.float32)

    in_maps = []
    for c in range(N_CORES):
        bg, og = divmod(c, OG)
        # x slice -> [I, T*BL] -> [N_BLK, 128(p), KT, BLK]
        xc = x_seq[:, bg * BL:(bg + 1) * BL, :]        # [T, BL, I]
        xTc = xc.transpose(2, 0, 1).reshape(I, TBc)    # [I, TBc]
        xPc = np.ascontiguousarray(
            xTc.reshape(KT, 128, N_BLK, BLK).transpose(2, 1, 0, 3))

        w_c = W[og * OL:(og + 1) * OL, :]              # [OL, I]
        wp = np.ascontiguousarray(
            w_c.T.reshape(KT, 128, NOT, 128).transpose(1, 0, 2, 3))

        b_c = b[og * OL:(og + 1) * OL]
        shift = b_c / (1.0 - DECAY)
        thr = (THR - shift).reshape(NOT, 128).T        # [128, NOT]
        u0 = (-shift).reshape(NOT, 128).T
        thr_tile = np.ascontiguousarray(
            np.broadcast_to(thr[:, :, None], (128, NOT, BL)), dtype=np.float32)
        u0_tile = np.ascontiguousarray(
            np.broadcast_to(u0[:, :, None], (128, NOT, BL)), dtype=np.float32)

        in_maps.append({"xP": xPc, "w_packed": wp,
                        "thr": thr_tile, "u0": u0_tile})

    try:
        res = bass_utils.run_bass_kernel_spmd(nc, in_maps,
                                              core_ids=list(range(N_CORES)))
    except Exception:
        # transient device errors (NRT_EXEC_UNIT_UNRECOVERABLE) recover
        # on re-run
        res = bass_utils.run_bass_kernel_spmd(nc, in_maps,
                                              core_ids=list(range(N_CORES)))
    global LAST_RESULT
    LAST_RESULT = res

    # Assemble: oc[p, t, ot, bl] -> out[t, bg*BL+bl, og*OL + ot*128 + p]
    out = np.empty((T, B, O), dtype=np.float32)
    for c in range(N_CORES):
        bg, og = divmod(c, OG)
        oc = np.asarray(res.results[c]["out"], dtype=np.float32)
        out[:, bg * BL:(bg + 1) * BL, og * OL:(og + 1) * OL] = (
            oc.transpose(1, 3, 2, 0).reshape(T, BL, OL))
    return out


LAST_RESULT = None


# revision 22
# speedup vs baseline: 1.0486x; 1.0486x over previous
"""LIF layer (T=64, B=128, 2048->2048) on 8 trn2 NeuronCores.

Strategy: hybrid sharding, 4-way over batch x 2-way over out_dim.
Each core owns 32 batches and 1024 output channels, so per-core HBM
traffic is x:16MB + W:8.4MB + out:4.2MB -- under the f32r matmul floor
(262144 rows @ 1 cyc/row ~ 109us). Single-pass f32r GEMM (measured
spike rel-err ~1.5e-2, gate is 2e-2; the error is the PE's internal
fp22 truncation, operand pre-rounding does not change it).

Per core:
  GEMM  cur[o, t*32+b] = sum_i W[o,i] x[t,b,i]  (f32r, 8 ot-tiles of 128)
  SCAN  64 LIF steps on [128, 8, 32] tiles (DVE), 3 fused ops/step,
        reading cur straight out of a 4-bank PSUM tile (strided AP).
Bias folded into per-channel threshold/init via u = mem - b/(1-decay).

DMA layout notes (per-partition line size drives DMA rate):
  x   packed per block -> one DMA of 16KB/partition lines
  W   4 k-chunks interleaved with x0 on the sync queue (first to spin
      up); the next blocks' x follows in FIFO order so prefetches
      can't starve the W stream
  out bf16 spikes, one slab DMA per block (4KB/partition lines),
      dram laid out [128, T, NOT, BL] so slabs are contiguous
"""

import math
import os

import numpy as np

import concourse.bacc as bacc
import concourse.bass as bass
import concourse.mybir as mybir
import concourse.tile as tile
from concourse import bass_utils

# Problem constants (hardcoded per contract)
T, B, I, O = 64, 128, 2048, 2048
N_CORES = 8
GRID = os.environ.get("LIF_GRID", "4x2")
BG, OG = (4, 2) if GRID == "4x2" else (2, 4)  # batch-groups x out-groups
BL = B // BG               # batches per core
OL = O // OG               # out-channels per core
NOT = OL // 128            # ot-tiles
TBc = T * BL               # columns per core
KT = I // 128              # 16 k-tiles
BLK = 256 if GRID == "4x2" else 512  # columns per block (= 8 timesteps)
SPB = BLK // BL            # timesteps per block
N_BLK = TBc // BLK         # 8 blocks
OTPB = max(1, 512 // BLK)  # ot-tiles sharing one psum bank
TAU, THR = 2.0, 1.0
DECAY = math.exp(-1.0 / TAU)

F32 = mybir.dt.float32
F32R = mybir.dt.float32r
BF16 = mybir.dt.bfloat16
ALU = mybir.AluOpType

MODE = f"f32r_{GRID}_v4"

_cache = {}


def _build_nc():
    nc = bacc.Bacc(trn_type="TRN2", target_bir_lowering=False)

    # x packed per block: [N_BLK, 128(p), KT, BLK] -> 16KB/partition lines
    xP = nc.dram_tensor("xP", [N_BLK, 128, KT, BLK], F32R, kind="ExternalInput")
    w_packed = nc.dram_tensor("w_packed", [128, KT, NOT, 128], F32R,
                              kind="ExternalInput")
    thr_d = nc.dram_tensor("thr", [128, NOT, BL], F32, kind="ExternalInput")
    u0_d = nc.dram_tensor("u0", [128, NOT, BL], F32, kind="ExternalInput")
    # t-major per partition so a block's 8 steps are one contiguous slab
    out_d = nc.dram_tensor("out", [128, T, NOT, BL], BF16, kind="ExternalOutput")

    with tile.TileContext(nc) as tc:
        with (
            tc.tile_pool(name="wpool", bufs=1) as wpool,
            tc.tile_pool(name="xpool", bufs=3) as xpool,
            tc.tile_pool(name="state", bufs=1) as state,
            tc.tile_pool(name="spkpool", bufs=2) as spkpool,
            tc.tile_pool(name="psum", bufs=2, space="PSUM") as psum_pool,
        ):
            # The sync queue spins up first (~8.6us vs ~12us for the
            # others), so the critical W + x0 stream rides it alone,
            # interleaved by k-chunk so the first matmuls gate on ~2.6MB.
            # The next block(s)' x follows on the SAME queue: FIFO order
            # keeps those prefetches from stealing bandwidth from W.
            HS = 2 if GRID == "4x2" else 1   # head blocks prefetched on sync
            w_all = wpool.tile([128, KT, NOT, 128], F32R)
            xt0 = xpool.tile([128, KT, BLK], F32R, tag="xt", name="xt_0")
            for wc in range(4):
                nc.sync.dma_start(w_all[:, 4 * wc:4 * wc + 4],
                                  w_packed[:, 4 * wc:4 * wc + 4])
                nc.sync.dma_start(xt0[:, 4 * wc:4 * wc + 4],
                                  xP[0][:, 4 * wc:4 * wc + 4])
            head_xts = {}
            for blk in range(1, HS + 1):
                xt = xpool.tile([128, KT, BLK], F32R, tag="xt",
                                name=f"xt_{blk}")
                nc.sync.dma_start(xt[:], xP[blk])
                head_xts[blk] = xt

            # also WARMS the scalar/gpsimd DMA queues: a queue's first use
            # pays ~12us of lazy ring init, so give each one a tiny DMA at
            # the head rather than letting a mid-run x prefetch eat it.
            u = state.tile([128, NOT, BL], F32)
            thr_t = state.tile([128, NOT, BL], F32)
            nc.scalar.dma_start(u[:], u0_d[:])
            nc.gpsimd.dma_start(thr_t[:], thr_d[:])

            for blk in range(N_BLK):
                if blk == 0:
                    xt = xt0
                elif blk in head_xts:
                    xt = head_xts[blk]
                else:
                    xt = xpool.tile([128, KT, BLK], F32R, tag="xt",
                                    name=f"xt_{blk}")
                    # alternate queues so consecutive prefetches overlap
                    eng = nc.scalar if blk % 2 else nc.gpsimd
                    eng.dma_start(xt[:], xP[blk])
                # 4-bank psum tile; each matmul dst sits in a bank half
                ps = psum_pool.tile([128, NOT, BLK], F32, tag="ps",
                                    name=f"ps_{blk}")
                for k in range(KT):
                    for ot in range(NOT):
                        # start=True zeroes the WHOLE psum bank; when a
                        # bank holds several ot tiles only the first one
                        # starts -- the others land on the bank-wide zero.
                        nc.tensor.matmul(
                            ps[:, ot, :],
                            w_all[:, k, ot, :],
                            xt[:, k, :],
                            start=(k == 0 and ot % OTPB == 0),
                            stop=(k == KT - 1),
                            skip_group_check=(ot % OTPB != 0),
                        )

                # LIF steps consuming this block's PSUM
                last_blk = blk == N_BLK - 1
                spk_blk = spkpool.tile([128, SPB, NOT, BL], BF16, tag="spk")
                for tl in range(SPB):
                    cur = ps[:, :, tl * BL:(tl + 1) * BL]  # [128, 8, 32]
                    nc.vector.scalar_tensor_tensor(
                        u[:], u[:], DECAY, cur, op0=ALU.mult, op1=ALU.add)
                    spk = spk_blk[:, tl]
                    nc.vector.tensor_tensor(spk, u[:], thr_t[:], op=ALU.is_gt)
                    if not (last_blk and tl == SPB - 1):
                        # u after the final step is unused
                        nc.vector.tensor_tensor(u[:], u[:], spk, op=ALU.subtract)
                    if last_blk:
                        # per-step DMAs so the final drain is one tiny
                        # transfer instead of a whole slab
                        nc.sync.dma_start(out_d[:, blk * SPB + tl], spk)
                if not last_blk:
                    # one slab DMA per block: [128, 8, NOT, BL] -> 4KB
                    # lines on the sync queue, which is idle after the head
                    nc.sync.dma_start(
                        out_d[:, blk * SPB:(blk + 1) * SPB], spk_blk[:])

    nc.compile()
    return nc


def _get_nc():
    if "nc" not in _cache:
        _cache["nc"] = _build_nc()
    return _cache["nc"]


def kernel(x_seq: np.ndarray, W: np.ndarray, b: np.ndarray) -> np.ndarray:
    nc = _get_nc()

    x_seq = np.asarray(x_seq, np.float32)
    W = np.asarray(W, np.float32)
    b = np.asarray(b, np.float32)

    in_maps = []
    for c in range(N_CORES):
        bg, og = divmod(c, OG)
        # x slice -> [I, T*BL] -> [N_BLK, 128(p), KT, BLK]
        xc = x_seq[:, bg * BL:(bg + 1) * BL, :]        # [T, BL, I]
        xTc = xc.transpose(2, 0, 1).reshape(I, TBc)    # [I, TBc]
        xPc = np.ascontiguousarray(
            xTc.reshape(KT, 128, N_BLK, BLK).transpose(2, 1, 0, 3))

        w_c = W[og * OL:(og + 1) * OL, :]              # [OL, I]
        wp = np.ascontiguousarray(
            w_c.T.reshape(KT, 128, NOT, 128).transpose(1, 0, 2, 3))

        b_c = b[og * OL:(og + 1) * OL]
        shift = b_c / (1.0 - DECAY)
        thr = (THR - shift).reshape(NOT, 128).T        # [128, NOT]
        u0 = (-shift).reshape(NOT, 128).T
        thr_tile = np.ascontiguousarray(
            np.broadcast_to(thr[:, :, None], (128, NOT, BL)), dtype=np.float32)
        u0_tile = np.ascontiguousarray(
            np.broadcast_to(u0[:, :, None], (128, NOT, BL)), dtype=np.float32)

        in_maps.append({"xP": xPc, "w_packed": wp,
                        "thr": thr_tile, "u0": u0_tile})

    try:
        res = bass_utils.run_bass_kernel_spmd(nc, in_maps,
                                              core_ids=list(range(N_CORES)))
    except Exception:
        # transient device errors (NRT_EXEC_UNIT_UNRECOVERABLE) recover
        # on re-run
        res = bass_utils.run_bass_kernel_spmd(nc, in_maps,
                                              core_ids=list(range(N_CORES)))
    global LAST_RESULT
    LAST_RESULT = res

    # Assemble: oc[p, t, ot, bl] -> out[t, bg*BL+bl, og*OL + ot*128 + p]
    out = np.empty((T, B, O), dtype=np.float32)
    for c in range(N_CORES):
        bg, og = divmod(c, OG)
        oc = np.asarray(res.results[c]["out"], dtype=np.float32)
        out[:, bg * BL:(bg + 1) * BL, og * OL:(og + 1) * OL] = (
            oc.transpose(1, 3, 2, 0).reshape(T, BL, OL))
    return out


LAST_RESULT = None


# revision 26
# speedup vs baseline: 1.1187x; 1.0669x over previous
"""LIF layer (T=64, B=128, 2048->2048) on 8 trn2 NeuronCores.

Strategy: hybrid sharding, 4-way over batch x 2-way over out_dim.
Each core owns 32 batches and 1024 output channels, so per-core HBM
traffic is x:16MB + W:8.4MB + out:4.2MB -- under the f32r matmul floor
(262144 rows @ 1 cyc/row ~ 109us). Single-pass f32r GEMM (measured
spike rel-err ~1.5e-2, gate is 2e-2; the error is the PE's internal
fp22 truncation, operand pre-rounding does not change it).

Per core:
  GEMM  cur[o, t*32+b] = sum_i W[o,i] x[t,b,i]  (f32r, 8 ot-tiles of 128)
  SCAN  64 LIF steps on [128, 8, 32] tiles (DVE), 3 fused ops/step,
        reading cur straight out of a 4-bank PSUM tile (strided AP).
Bias folded into per-channel threshold/init via u = mem - b/(1-decay).

DMA layout notes (per-partition line size drives DMA rate):
  x   packed per block -> one DMA of 16KB/partition lines
  W   4 k-chunks interleaved with x0 on the sync queue (first to spin
      up); the next blocks' x follows in FIFO order so prefetches
      can't starve the W stream
  out bf16 spikes, one slab DMA per block (4KB/partition lines),
      dram laid out [128, T, NOT, BL] so slabs are contiguous
"""

import math
import os

import numpy as np

import concourse.bacc as bacc
import concourse.bass as bass
import concourse.mybir as mybir
import concourse.tile as tile
from concourse import bass_utils

# Problem constants (hardcoded per contract)
T, B, I, O = 64, 128, 2048, 2048
N_CORES = 8
GRID = os.environ.get("LIF_GRID", "4x2")
BG, OG = (4, 2) if GRID == "4x2" else (2, 4)  # batch-groups x out-groups
BL = B // BG               # batches per core
OL = O // OG               # out-channels per core
NOT = OL // 128            # ot-tiles
TBc = T * BL               # columns per core
KT = I // 128              # 16 k-tiles
BLK = 256 if GRID == "4x2" else 512  # columns per block (= 8 timesteps)
SPB = BLK // BL            # timesteps per block
N_BLK = TBc // BLK         # 8 blocks
OTPB = max(1, 512 // BLK)  # ot-tiles sharing one psum bank
TAU, THR = 2.0, 1.0
DECAY = math.exp(-1.0 / TAU)

F32 = mybir.dt.float32
F32R = mybir.dt.float32r
BF16 = mybir.dt.bfloat16
ALU = mybir.AluOpType

SCANSPLIT = os.environ.get("LIF_SCANSPLIT", "0") == "1"
MODE = f"f32r_{GRID}_v5" + ("_ss" if SCANSPLIT else "")

_cache = {}


def _build_nc():
    nc = bacc.Bacc(trn_type="TRN2", target_bir_lowering=False)

    # x packed per block: [N_BLK, 128(p), KT, BLK] -> 16KB/partition lines
    xP = nc.dram_tensor("xP", [N_BLK, 128, KT, BLK], F32R, kind="ExternalInput")
    w_packed = nc.dram_tensor("w_packed", [128, KT, NOT, 128], F32R,
                              kind="ExternalInput")
    thr_d = nc.dram_tensor("thr", [128, NOT, BL], F32, kind="ExternalInput")
    u0_d = nc.dram_tensor("u0", [128, NOT, BL], F32, kind="ExternalInput")
    # t-major per partition so a block's 8 steps are one contiguous slab
    out_d = nc.dram_tensor("out", [128, T, NOT, BL], BF16, kind="ExternalOutput")

    with tile.TileContext(nc) as tc:
        with (
            tc.tile_pool(name="wpool", bufs=1) as wpool,
            tc.tile_pool(name="xpool", bufs=3) as xpool,
            tc.tile_pool(name="state", bufs=1) as state,
            tc.tile_pool(name="spkpool", bufs=2) as spkpool,
            tc.tile_pool(name="psum", bufs=2, space="PSUM") as psum_pool,
        ):
            # The sync queue spins up first (~8.6us vs ~12us for the
            # others), so the critical W + x0 stream rides it alone,
            # interleaved by k-chunk so the first matmuls gate on ~2.6MB.
            # The next block(s)' x follows on the SAME queue: FIFO order
            # keeps those prefetches from stealing bandwidth from W.
            HS = 2 if GRID == "4x2" else 1   # head blocks prefetched on sync
            w_all = wpool.tile([128, KT, NOT, 128], F32R)
            xt0 = xpool.tile([128, KT, BLK], F32R, tag="xt", name="xt_0")
            for wc in range(4):
                nc.sync.dma_start(w_all[:, 4 * wc:4 * wc + 4],
                                  w_packed[:, 4 * wc:4 * wc + 4])
                nc.sync.dma_start(xt0[:, 4 * wc:4 * wc + 4],
                                  xP[0][:, 4 * wc:4 * wc + 4])
            head_xts = {0: xt0}
            for blk in range(1, HS + 1):
                xt = xpool.tile([128, KT, BLK], F32R, tag="xt",
                                name=f"xt_{blk}")
                nc.sync.dma_start(xt[:], xP[blk])
                head_xts[blk] = xt

            # also WARMS the scalar/gpsimd DMA queues: a queue's first use
            # pays ~12us of lazy ring init, so give each one a tiny DMA at
            # the head rather than letting a mid-run x prefetch eat it.
            u = state.tile([128, NOT, BL], F32)
            thr_t = state.tile([128, NOT, BL], F32)
            nc.scalar.dma_start(u[:], u0_d[:])
            nc.gpsimd.dma_start(thr_t[:], thr_d[:])

            for blk in range(N_BLK):
                if blk == 0:
                    xt = xt0
                elif blk in head_xts:
                    xt = head_xts[blk]
                else:
                    xt = xpool.tile([128, KT, BLK], F32R, tag="xt",
                                    name=f"xt_{blk}")
                    # alternate queues so consecutive prefetches overlap
                    eng = nc.scalar if blk % 2 else nc.gpsimd
                    eng.dma_start(xt[:], xP[blk])
                # 4-bank psum tile; each matmul dst sits in a bank half
                ps = psum_pool.tile([128, NOT, BLK], F32, tag="ps",
                                    name=f"ps_{blk}")
                for k in range(KT):
                    for ot in range(NOT):
                        # start=True zeroes the WHOLE psum bank; when a
                        # bank holds several ot tiles only the first one
                        # starts -- the others land on the bank-wide zero.
                        nc.tensor.matmul(
                            ps[:, ot, :],
                            w_all[:, k, ot, :],
                            xt[:, k, :],
                            start=(k == 0 and ot % OTPB == 0),
                            stop=(k == KT - 1),
                            skip_group_check=(ot % OTPB != 0),
                        )

                # LIF steps consuming this block's PSUM
                last_blk = blk == N_BLK - 1
                spk_blk = spkpool.tile([128, SPB, NOT, BL], BF16, tag="spk")
                H = NOT // 2
                for tl in range(SPB):
                    cur = ps[:, :, tl * BL:(tl + 1) * BL]  # [128, 8, 32]
                    spk = spk_blk[:, tl]
                    last_step = last_blk and tl == SPB - 1
                    if SCANSPLIT:
                        # ot-halves on two engines: DVE owns the psum-
                        # reading fma for both halves (gpsimd has no PSUM
                        # port); gt+sub of the upper half run on gpsimd in
                        # parallel with the lower half on DVE.
                        nc.vector.scalar_tensor_tensor(
                            u[:, H:], u[:, H:], DECAY, cur[:, H:],
                            op0=ALU.mult, op1=ALU.add)
                        nc.gpsimd.tensor_tensor(
                            spk[:, H:], u[:, H:], thr_t[:, H:], op=ALU.is_gt)
                        nc.vector.scalar_tensor_tensor(
                            u[:, :H], u[:, :H], DECAY, cur[:, :H],
                            op0=ALU.mult, op1=ALU.add)
                        nc.vector.tensor_tensor(
                            spk[:, :H], u[:, :H], thr_t[:, :H], op=ALU.is_gt)
                        if not last_step:
                            nc.gpsimd.tensor_tensor(
                                u[:, H:], u[:, H:], spk[:, H:],
                                op=ALU.subtract)
                            nc.vector.tensor_tensor(
                                u[:, :H], u[:, :H], spk[:, :H],
                                op=ALU.subtract)
                    else:
                        nc.vector.scalar_tensor_tensor(
                            u[:], u[:], DECAY, cur, op0=ALU.mult, op1=ALU.add)
                        nc.vector.tensor_tensor(spk, u[:], thr_t[:],
                                                op=ALU.is_gt)
                        if not last_step:
                            # u after the final step is unused
                            nc.vector.tensor_tensor(u[:], u[:], spk,
                                                    op=ALU.subtract)
                    if last_blk:
                        # per-step DMAs so the final drain is one tiny
                        # transfer instead of a whole slab
                        nc.sync.dma_start(out_d[:, blk * SPB + tl], spk)
                if not last_blk:
                    # one slab DMA per block: [128, 8, NOT, BL] -> 4KB
                    # lines on the sync queue, which is idle after the head
                    nc.sync.dma_start(
                        out_d[:, blk * SPB:(blk + 1) * SPB], spk_blk[:])

    nc.compile()
    return nc


def _get_nc():
    if "nc" not in _cache:
        _cache["nc"] = _build_nc()
    return _cache["nc"]


def kernel(x_seq: np.ndarray, W: np.ndarray, b: np.ndarray) -> np.ndarray:
    nc = _get_nc()

    x_seq = np.asarray(x_seq, np.float32)
    W = np.asarray(W, np.float32)
    b = np.asarray(b, np.float32)

    in_maps = []
    for c in range(N_CORES):
        bg, og = divmod(c, OG)
        # x slice -> [I, T*BL] -> [N_BLK, 128(p), KT, BLK]
        xc = x_seq[:, bg * BL:(bg + 1) * BL, :]        # [T, BL, I]
        xTc = xc.transpose(2, 0, 1).reshape(I, TBc)    # [I, TBc]
        xPc = np.ascontiguousarray(
            xTc.reshape(KT, 128, N_BLK, BLK).transpose(2, 1, 0, 3))

        w_c = W[og * OL:(og + 1) * OL, :]              # [OL, I]
        wp = np.ascontiguousarray(
            w_c.T.reshape(KT, 128, NOT, 128).transpose(1, 0, 2, 3))

        b_c = b[og * OL:(og + 1) * OL]
        shift = b_c / (1.0 - DECAY)
        thr = (THR - shift).reshape(NOT, 128).T        # [128, NOT]
        u0 = (-shift).reshape(NOT, 128).T
        thr_tile = np.ascontiguousarray(
            np.broadcast_to(thr[:, :, None], (128, NOT, BL)), dtype=np.float32)
        u0_tile = np.ascontiguousarray(
            np.broadcast_to(u0[:, :, None], (128, NOT, BL)), dtype=np.float32)

        in_maps.append({"xP": xPc, "w_packed": wp,
                        "thr": thr_tile, "u0": u0_tile})

    try:
        res = bass_utils.run_bass_kernel_spmd(nc, in_maps,
                                              core_ids=list(range(N_CORES)))
    except Exception:
        # transient device errors (NRT_EXEC_UNIT_UNRECOVERABLE) recover
        # on re-run
        res = bass_utils.run_bass_kernel_spmd(nc, in_maps,
                                              core_ids=list(range(N_CORES)))
    global LAST_RESULT
    LAST_RESULT = res

    # Assemble: oc[p, t, ot, bl] -> out[t, bg*BL+bl, og*OL + ot*128 + p]
    out = np.empty((T, B, O), dtype=np.float32)
    for c in range(N_CORES):
        bg, og = divmod(c, OG)
        oc = np.asarray(res.results[c]["out"], dtype=np.float32)
        out[:, bg * BL:(bg + 1) * BL, og * OL:(og + 1) * OL] = (
            oc.transpose(1, 3, 2, 0).reshape(T, BL, OL))
    return out


LAST_RESULT = None
